# revision 11
# baseline (speedup 1.0000x reference)
"""Int4-packed linear (group-quantized, 256-group) on 8 Trainium2 cores.

Column-parallel: each core owns 1024 of 8192 out_features.

Math per core (out^T orientation, o on partitions):
  out[o, t] = sum_g s[o,g] * R_g[o,t] + corr[o,t]
  R_g[o,t]  = sum_{i in g} q[o,i] * x[t,i]        (q in 0..15)
  corr[o,t] = -8*sum_g s[o,g]*xsum_g[t] + bias[o]   (precomputed on host)

Weights ship as fp8e4m3 nibble planes (exact small integers), x as bf16,
all streaming on the Sync HWDGE ring at the per-core HBM roofline.

The combine is engineered around measured engine limits: exec time is
set by DVE's total work (reduce is DVE-only, and any concurrent Pool op
knocks DVE's 2x SBUF mode down to 1x), so the group reduction runs on
the idle TensorEngine instead: after ACT extracts R and DVE applies the
scales (one 2x multiply per tile), 8 identity-lhsT matmuls accumulate
the 32 scaled group slices into PSUM as quad-sums [o, t, 4]. DVE then
only does a tiny reduce4 + corr add per tile (~0.8 us instead of ~2 us
of fold+reduce17). Pool does nothing, preserving DVE's 2x mode. The
static per-engine order is forced with tile_set_cur_wait stamps.
o-tile 7 arrives last as 16/8/8-group pieces with a short finish chain.
"""

import sys

import numpy as np
import ml_dtypes

sys.path.insert(0, "/opt/trn_rl_repo")

import concourse.bass as bass  # noqa: E402
import concourse.mybir as mybir  # noqa: E402
import concourse.tile as tile  # noqa: E402
from concourse import bacc  # noqa: E402

NCORES = 8
TOKENS = 64
IN_F = 8192
OUT_F = 8192
GROUP = 256
OC = OUT_F // NCORES  # 1024 out-features per core
NCHUNK = IN_F // 128  # 64 K-chunks of 128
NG = IN_F // GROUP  # 32 groups
NH = NG // 2  # 16 groups per PSUM half tile
NOT = OC // 128  # 8 o-tiles per core

_cache = {}


def _build_nc():
    if "nc" in _cache:
        return _cache["nc"], _cache["names"]

    f32 = mybir.dt.float32
    bf16 = mybir.dt.bfloat16
    nc = bacc.Bacc(None, target_bir_lowering=False, debug=False)
    with tile.TileContext(nc) as tc:
        stamp_n = [0]

        def stamp():
            # Monotone scheduler-sim timestamps: forces the committed
            # per-engine static order to equal emission order.
            stamp_n[0] += 1
            tc.tile_set_cur_wait(0.002 * stamp_n[0])

        with tc.tile_pool(name="dram", bufs=1, space="DRAM") as dram:
            # w8[ot, p, r, c] = nibble_fp8[ot*128 + c, 128*r + p]
            w8 = dram.tile([NOT, 128, NCHUNK, 128], mybir.dt.float8e4,
                           kind="ExternalInput")
            xt = dram.tile([128, NCHUNK, TOKENS], bf16, kind="ExternalInput")
            # scco[:, ot, 0:32] = scales, scco[:, ot, 32:96] = corr
            scco = dram.tile([128, NOT, NG + TOKENS], bf16,
                             kind="ExternalInput")
            ident = dram.tile([128, 128], bf16, kind="ExternalInput")
            outT = dram.tile([128, NOT, TOKENS], bf16, kind="ExternalOutput")

            with (
                tc.tile_pool(name="wsb", bufs=1) as wsb,
                tc.tile_pool(name="xsb", bufs=1) as xsb,
                tc.tile_pool(name="small", bufs=1) as small,
                tc.tile_pool(name="cmb", bufs=1) as cmb,
                tc.tile_pool(name="ps", bufs=2, space="PSUM") as ps,
            ):
                NF = NOT - 1  # tiles 0-6 full; o-tile 7 pieced at the end
                w_all = wsb.tile([128, NOT, NCHUNK, 128], mybir.dt.float8e4)
                x_all = xsb.tile([128, NCHUNK, TOKENS], bf16)
                scco_sb = small.tile([128, NOT, NG + TOKENS], bf16)
                id_sb = small.tile([128, 128], bf16, tag="id")
                raw_all = cmb.tile([128, NOT, TOKENS, NG], bf16)
                scl_all = cmb.tile([128, NOT, TOKENS, NG], bf16, tag="scl")
                tmp_all = cmb.tile([128, NOT, TOKENS], bf16, tag="tmp")
                y_all = cmb.tile([128, NOT, TOKENS], bf16, tag="y")

                # Everything on the Sync ring, in consumption order.
                stamp()
                nc.sync.dma_start(out=id_sb[:], in_=ident[:])
                stamp()
                nc.sync.dma_start(out=scco_sb[:], in_=scco[:])
                stamp()
                nc.sync.dma_start(out=x_all[:], in_=xt[:])
                for ot in range(NF):
                    stamp()
                    nc.sync.dma_start(out=w_all[:, ot, 0:32, :],
                                      in_=w8[ot, :, 0:32, :])
                    stamp()
                    nc.sync.dma_start(out=w_all[:, ot, 32:64, :],
                                      in_=w8[ot, :, 32:64, :])
                for c0, c1 in ((0, 32), (32, 48), (48, 64)):
                    stamp()
                    nc.sync.dma_start(out=w_all[:, NOT - 1, c0:c1, :],
                                      in_=w8[NOT - 1, :, c0:c1, :])

                def mm_groups(ot, glo, ghi, r_ps):
                    # start once per 2 KB PSUM bank (8 slots of 64 fp32)
                    n = ghi - glo
                    for gg in range(n):
                        g = glo + gg
                        stamp()
                        nc.tensor.matmul(
                            r_ps[:, gg, :],
                            lhsT=w_all[:, ot, 2 * g, :],
                            rhs=x_all[:, 2 * g, :],
                            start=(gg % 8 == 0),
                            stop=False,
                        )
                        stamp()
                        nc.tensor.matmul(
                            r_ps[:, gg, :],
                            lhsT=w_all[:, ot, 2 * g + 1, :],
                            rhs=x_all[:, 2 * g + 1, :],
                            start=False,
                            stop=(gg == n - 1 or gg % 8 == 7),
                        )

                def psum_tg(r_ps, npg):
                    # PSUM [o, g, t] viewed as [o, t, g]
                    return bass.AP(
                        tensor=r_ps.tensor,
                        offset=r_ps.offset,
                        ap=[r_ps.ap[0], [1, TOKENS], [TOKENS, npg]],
                    )

                def s_bc(ot, lo, n):
                    # s[o, g] broadcast along t
                    s_ot = scco_sb[:, ot, :]
                    return bass.AP(
                        tensor=s_ot.tensor,
                        offset=s_ot.offset + lo,
                        ap=[s_ot.ap[0], [0, TOKENS], [1, n]],
                    )

                mul = mybir.AluOpType.mult
                add = mybir.AluOpType.add
                X = mybir.AxisListType.X

                def mkps(n, tag, bufs):
                    return ps.tile([128, n, TOKENS], f32, tag=tag,
                                   bufs=bufs, name=tag)

                def emit_ymm(ot, qlo, qhi, py):
                    # TensorE: accumulate scaled group slices into quad
                    # sums: py[o, 4t+q] += scl[o, t, 4j+q] for j in quads
                    for j in range(qlo, qhi):
                        stamp()
                        nc.tensor.matmul(
                            py[:, :, :],
                            lhsT=id_sb[:],
                            rhs=scl_all[:, ot, :, 4 * j:4 * j + 4],
                            start=(j == 0),
                            stop=(j == 7),
                        )

                def emit_finish(ot, py):
                    # DVE: reduce the 4 quad sums + add corr
                    stamp()
                    nc.vector.tensor_reduce(
                        out=tmp_all[:, ot, :],
                        in_=bass.AP(tensor=py.tensor, offset=py.offset,
                                    ap=[py.ap[0], [4, TOKENS], [1, 4]]),
                        axis=X, op=add)
                    stamp()
                    nc.vector.tensor_tensor(
                        out=y_all[:, ot, :], in0=tmp_all[:, ot, :],
                        in1=scco_sb[:, ot, NG:], op=add)

                with nc.allow_low_precision("bf16 combine, validated vs 2e-2"):
                    pys = {}
                    for ot in range(NF):
                        rA = mkps(NH, "rA", 2)
                        mm_groups(ot, 0, NH, rA)
                        rB = mkps(NH, "rB", 1)
                        mm_groups(ot, NH, NG, rB)
                        if ot >= 1:
                            emit_ymm(ot - 1, 0, 8, pys[ot - 1])
                        raw = raw_all[:, ot, :, :]
                        stamp()
                        nc.scalar.copy(out=raw[:, :, 0:NH],
                                       in_=psum_tg(rA, NH))
                        stamp()
                        nc.scalar.copy(out=raw[:, :, NH:NG],
                                       in_=psum_tg(rB, NH))
                        stamp()
                        nc.vector.tensor_tensor(
                            out=scl_all[:, ot, :, :], in0=raw[:],
                            in1=s_bc(ot, 0, NG), op=mul)
                        pys[ot] = mkps(4, "py", 2)
                        if ot >= 2:
                            emit_finish(ot - 2, pys[ot - 2])

                    # drain tiles 5 and 6 of the software pipeline
                    emit_ymm(NF - 1, 0, 8, pys[NF - 1])
                    emit_finish(NF - 2, pys[NF - 2])
                    emit_finish(NF - 1, pys[NF - 1])
                    stamp()
                    nc.sync.dma_start(out=outT[:, 0:NF, :],
                                      in_=y_all[:, 0:NF, :])

                    # ---- o-tile 7: A(g0-15) + B1(g16-23) + B2(g24-31) ----
                    L7 = NOT - 1
                    py7 = mkps(4, "py", 2)
                    rA7 = mkps(NH, "rA", 2)
                    mm_groups(L7, 0, 16, rA7)
                    stamp()
                    nc.scalar.copy(out=raw_all[:, L7, :, 0:16],
                                   in_=psum_tg(rA7, 16))
                    stamp()
                    nc.vector.tensor_tensor(
                        out=scl_all[:, L7, :, 0:16],
                        in0=raw_all[:, L7, :, 0:16],
                        in1=s_bc(L7, 0, 16), op=mul)
                    emit_ymm(L7, 0, 4, py7)

                    rB1 = mkps(8, "rB", 1)
                    mm_groups(L7, 16, 24, rB1)
                    stamp()
                    nc.scalar.copy(out=raw_all[:, L7, :, 16:24],
                                   in_=psum_tg(rB1, 8))
                    stamp()
                    nc.vector.tensor_tensor(
                        out=scl_all[:, L7, :, 16:24],
                        in0=raw_all[:, L7, :, 16:24],
                        in1=s_bc(L7, 16, 8), op=mul)
                    emit_ymm(L7, 4, 6, py7)

                    rB2 = mkps(8, "rB", 1)
                    mm_groups(L7, 24, 32, rB2)
                    stamp()
                    nc.scalar.copy(out=raw_all[:, L7, :, 24:32],
                                   in_=psum_tg(rB2, 8))
                    stamp()
                    nc.vector.tensor_tensor(
                        out=scl_all[:, L7, :, 24:32],
                        in0=raw_all[:, L7, :, 24:32],
                        in1=s_bc(L7, 24, 8), op=mul)
                    emit_ymm(L7, 6, 8, py7)

                    # token-split finish + stores
                    stamp()
                    nc.vector.tensor_reduce(
                        out=tmp_all[:, L7, :],
                        in_=bass.AP(tensor=py7.tensor, offset=py7.offset,
                                    ap=[py7.ap[0], [4, TOKENS], [1, 4]]),
                        axis=X, op=add)
                    for t0, t1 in ((0, TOKENS // 2), (TOKENS // 2, TOKENS)):
                        stamp()
                        nc.vector.tensor_tensor(
                            out=y_all[:, L7, t0:t1],
                            in0=tmp_all[:, L7, t0:t1],
                            in1=scco_sb[:, L7, NG + t0:NG + t1], op=add)
                        stamp()
                        nc.sync.dma_start(out=outT[:, L7, t0:t1],
                                          in_=y_all[:, L7, t0:t1])

    nc.compile()
    names = dict(w8=w8.name, xt=xt.name, scco=scco.name, ident=ident.name,
                 outT=outT.name)
    _cache["nc"] = nc
    _cache["names"] = names
    return nc, names


def _gather_core(outT_host):
    # outT_host[p, ot, t] -> [t, ot*128 + p]
    o = np.asarray(outT_host).astype(np.float32)
    return o.transpose(2, 1, 0).reshape(TOKENS, OC)


def _host_prep(x, weight_packed, scales, bias):
    """Build the 8 per-core input maps."""
    _, names = _build_nc()

    wp = np.ascontiguousarray(weight_packed).view(np.uint32)  # [8192, 1024]
    shifts = (np.arange(8, dtype=np.uint32) * 4)[None, None, :]
    nib = ((wp[:, :, None] >> shifts) & np.uint32(0xF)).astype(np.uint8)
    nib = nib.reshape(OUT_F, IN_F)  # n[o, i]
    lut = np.arange(16, dtype=np.float32).astype(ml_dtypes.float8_e4m3)
    nfp8 = lut[nib]  # [8192, 8192] fp8, exact

    xb = x.astype(ml_dtypes.bfloat16)
    xf = xb.astype(np.float32)
    # xt_host[p, r, t] = x_bf16[t, 128r + p]
    xt_host = np.ascontiguousarray(
        xb.T.reshape(NCHUNK, 128, TOKENS).transpose(1, 0, 2))
    # corr[o, t] = -8 * sum_g s[o,g] * xsum_g[t] + bias[o]
    xsum = xf.reshape(TOKENS, NG, GROUP).sum(axis=2)  # [t, g]
    corr = (-8.0 * scales.astype(np.float64) @ xsum.astype(np.float64).T
            + bias.astype(np.float64)[:, None]).astype(np.float32)  # [8192, 64]
    ident_host = np.eye(128, dtype=ml_dtypes.bfloat16)

    in_maps = []
    for k in range(NCORES):
        osl = slice(OC * k, OC * (k + 1))
        nk = nfp8[osl]  # [1024, 8192]
        # w8_host[ot, p, r, c] = nk[ot*128 + c, 128*r + p]
        w8_host = np.ascontiguousarray(
            nk.reshape(NOT, 128, NCHUNK, 128).transpose(0, 3, 2, 1)
        )
        sck = scales[osl]  # [1024, 32]
        scco_host = np.empty((128, NOT, NG + TOKENS), dtype=ml_dtypes.bfloat16)
        scco_host[:, :, :NG] = sck.reshape(NOT, 128, NG).transpose(1, 0, 2)
        scco_host[:, :, NG:] = corr[osl].reshape(NOT, 128, TOKENS).transpose(
            1, 0, 2)
        in_maps.append({
            names["w8"]: w8_host,
            names["xt"]: xt_host,
            names["scco"]: np.ascontiguousarray(scco_host),
            names["ident"]: ident_host,
        })
    return in_maps


def kernel(x, weight_packed, scales, bias):
    from concourse.bass_utils import run_bass_kernel_spmd

    nc, names = _build_nc()
    in_maps = _host_prep(x, weight_packed, scales, bias)
    res = run_bass_kernel_spmd(nc, in_maps, core_ids=list(range(NCORES)))
    out = np.concatenate(
        [_gather_core(res.results[k][names["outT"]]) for k in range(NCORES)],
        axis=1,
    )  # [64, 8192]
    return np.ascontiguousarray(out)


# revision 12
# speedup vs baseline: 1.0276x; 1.0276x over previous
"""Int4-packed linear (group-quantized, 256-group) on 8 Trainium2 cores.

Column-parallel: each core owns 1024 of 8192 out_features.

Math per core (out^T orientation, o on partitions):
  out[o, t] = sum_g s[o,g] * R_g[o,t] + corr[o,t]
  R_g[o,t]  = sum_{i in g} q[o,i] * x[t,i]        (q in 0..15)
  corr[o,t] = -8*sum_g s[o,g]*xsum_g[t] + bias[o]   (precomputed on host)

Weights ship as fp8e4m3 nibble planes (exact small integers), x as bf16,
all streaming on the Sync HWDGE ring at the per-core HBM roofline.

The combine is engineered around measured engine limits: exec time is
set by DVE's total work (reduce is DVE-only, and any concurrent Pool op
knocks DVE's 2x SBUF mode down to 1x), so the group reduction runs on
the idle TensorEngine instead: after ACT extracts R and DVE applies the
scales (one 2x multiply per tile), 8 identity-lhsT matmuls accumulate
the 32 scaled group slices into PSUM as quad-sums [o, t, 4]. DVE then
only does a tiny reduce4 + corr add per tile (~0.8 us instead of ~2 us
of fold+reduce17). Pool does nothing, preserving DVE's 2x mode. The
static per-engine order is forced with tile_set_cur_wait stamps.
o-tile 7 arrives last as 16/8/8-group pieces with a short finish chain.
"""

import sys

import numpy as np
import ml_dtypes

sys.path.insert(0, "/opt/trn_rl_repo")

import concourse.bass as bass  # noqa: E402
import concourse.mybir as mybir  # noqa: E402
import concourse.tile as tile  # noqa: E402
from concourse import bacc  # noqa: E402

NCORES = 8
TOKENS = 64
IN_F = 8192
OUT_F = 8192
GROUP = 256
OC = OUT_F // NCORES  # 1024 out-features per core
NCHUNK = IN_F // 128  # 64 K-chunks of 128
NG = IN_F // GROUP  # 32 groups
NH = NG // 2  # 16 groups per PSUM half tile
NOT = OC // 128  # 8 o-tiles per core

_cache = {}


def _build_nc():
    if "nc" in _cache:
        return _cache["nc"], _cache["names"]

    f32 = mybir.dt.float32
    bf16 = mybir.dt.bfloat16
    nc = bacc.Bacc(None, target_bir_lowering=False, debug=False)
    with tile.TileContext(nc) as tc:
        stamp_n = [0]

        def stamp():
            # Monotone scheduler-sim timestamps: forces the committed
            # per-engine static order to equal emission order.
            stamp_n[0] += 1
            tc.tile_set_cur_wait(0.002 * stamp_n[0])

        with tc.tile_pool(name="dram", bufs=1, space="DRAM") as dram:
            # w8[p, ot, r, c] = nibble_fp8[ot*128 + c, 128*r + p]
            # (partition-major: strided per-transfer reads spread across
            # HBM channels ~20% faster than contiguous o-tile regions)
            w8 = dram.tile([128, NOT, NCHUNK, 128], mybir.dt.float8e4,
                           kind="ExternalInput")
            xt = dram.tile([128, NCHUNK, TOKENS], bf16, kind="ExternalInput")
            # scco[:, ot, 0:32] = scales, scco[:, ot, 32:96] = corr
            scco = dram.tile([128, NOT, NG + TOKENS], bf16,
                             kind="ExternalInput")
            ident = dram.tile([128, 128], bf16, kind="ExternalInput")
            outT = dram.tile([128, NOT, TOKENS], bf16, kind="ExternalOutput")

            with (
                tc.tile_pool(name="wsb", bufs=1) as wsb,
                tc.tile_pool(name="xsb", bufs=1) as xsb,
                tc.tile_pool(name="small", bufs=1) as small,
                tc.tile_pool(name="cmb", bufs=1) as cmb,
                tc.tile_pool(name="ps", bufs=2, space="PSUM") as ps,
            ):
                NF = NOT - 1  # tiles 0-6 full; o-tile 7 pieced at the end
                w_all = wsb.tile([128, NOT, NCHUNK, 128], mybir.dt.float8e4)
                x_all = xsb.tile([128, NCHUNK, TOKENS], bf16)
                scco_sb = small.tile([128, NOT, NG + TOKENS], bf16)
                id_sb = small.tile([128, 128], bf16, tag="id")
                raw_all = cmb.tile([128, NOT, TOKENS, NG], bf16)
                scl_all = cmb.tile([128, NOT, TOKENS, NG], bf16, tag="scl")
                tmp_all = cmb.tile([128, NOT, TOKENS], bf16, tag="tmp")
                y_all = cmb.tile([128, NOT, TOKENS], bf16, tag="y")

                # Everything on the Sync ring, in consumption order.
                stamp()
                nc.sync.dma_start(out=id_sb[:], in_=ident[:])
                stamp()
                nc.sync.dma_start(out=scco_sb[:], in_=scco[:])
                stamp()
                nc.sync.dma_start(out=x_all[:], in_=xt[:])
                for ot in range(NF):
                    stamp()
                    nc.sync.dma_start(out=w_all[:, ot, 0:32, :],
                                      in_=w8[:, ot, 0:32, :])
                    stamp()
                    nc.sync.dma_start(out=w_all[:, ot, 32:64, :],
                                      in_=w8[:, ot, 32:64, :])
                for c0, c1 in ((0, 32), (32, 48), (48, 64)):
                    stamp()
                    nc.sync.dma_start(out=w_all[:, NOT - 1, c0:c1, :],
                                      in_=w8[:, NOT - 1, c0:c1, :])

                def mm_groups(ot, glo, ghi, r_ps):
                    # start once per 2 KB PSUM bank (8 slots of 64 fp32)
                    n = ghi - glo
                    for gg in range(n):
                        g = glo + gg
                        stamp()
                        nc.tensor.matmul(
                            r_ps[:, gg, :],
                            lhsT=w_all[:, ot, 2 * g, :],
                            rhs=x_all[:, 2 * g, :],
                            start=(gg % 8 == 0),
                            stop=False,
                        )
                        stamp()
                        nc.tensor.matmul(
                            r_ps[:, gg, :],
                            lhsT=w_all[:, ot, 2 * g + 1, :],
                            rhs=x_all[:, 2 * g + 1, :],
                            start=False,
                            stop=(gg == n - 1 or gg % 8 == 7),
                        )

                def psum_tg(r_ps, npg):
                    # PSUM [o, g, t] viewed as [o, t, g]
                    return bass.AP(
                        tensor=r_ps.tensor,
                        offset=r_ps.offset,
                        ap=[r_ps.ap[0], [1, TOKENS], [TOKENS, npg]],
                    )

                def s_bc(ot, lo, n):
                    # s[o, g] broadcast along t
                    s_ot = scco_sb[:, ot, :]
                    return bass.AP(
                        tensor=s_ot.tensor,
                        offset=s_ot.offset + lo,
                        ap=[s_ot.ap[0], [0, TOKENS], [1, n]],
                    )

                mul = mybir.AluOpType.mult
                add = mybir.AluOpType.add
                X = mybir.AxisListType.X

                def mkps(n, tag, bufs):
                    return ps.tile([128, n, TOKENS], f32, tag=tag,
                                   bufs=bufs, name=tag)

                def emit_ymm(ot, qlo, qhi, py):
                    # TensorE: accumulate scaled group slices into quad
                    # sums: py[o, 4t+q] += scl[o, t, 4j+q] for j in quads
                    for j in range(qlo, qhi):
                        stamp()
                        nc.tensor.matmul(
                            py[:, :, :],
                            lhsT=id_sb[:],
                            rhs=scl_all[:, ot, :, 4 * j:4 * j + 4],
                            start=(j == 0),
                            stop=(j == 7),
                        )

                def emit_finish(ot, py):
                    # DVE: reduce the 4 quad sums + add corr
                    stamp()
                    nc.vector.tensor_reduce(
                        out=tmp_all[:, ot, :],
                        in_=bass.AP(tensor=py.tensor, offset=py.offset,
                                    ap=[py.ap[0], [4, TOKENS], [1, 4]]),
                        axis=X, op=add)
                    stamp()
                    nc.vector.tensor_tensor(
                        out=y_all[:, ot, :], in0=tmp_all[:, ot, :],
                        in1=scco_sb[:, ot, NG:], op=add)

                with nc.allow_low_precision("bf16 combine, validated vs 2e-2"):
                    pys = {}
                    for ot in range(NF):
                        rA = mkps(NH, "rA", 2)
                        mm_groups(ot, 0, NH, rA)
                        rB = mkps(NH, "rB", 1)
                        mm_groups(ot, NH, NG, rB)
                        if ot >= 1:
                            emit_ymm(ot - 1, 0, 8, pys[ot - 1])
                        raw = raw_all[:, ot, :, :]
                        stamp()
                        nc.scalar.copy(out=raw[:, :, 0:NH],
                                       in_=psum_tg(rA, NH))
                        stamp()
                        nc.scalar.copy(out=raw[:, :, NH:NG],
                                       in_=psum_tg(rB, NH))
                        stamp()
                        nc.vector.tensor_tensor(
                            out=scl_all[:, ot, :, :], in0=raw[:],
                            in1=s_bc(ot, 0, NG), op=mul)
                        pys[ot] = mkps(4, "py", 2)
                        if ot >= 2:
                            emit_finish(ot - 2, pys[ot - 2])

                    # drain tiles 5 and 6 of the software pipeline
                    emit_ymm(NF - 1, 0, 8, pys[NF - 1])
                    emit_finish(NF - 2, pys[NF - 2])
                    emit_finish(NF - 1, pys[NF - 1])
                    stamp()
                    nc.sync.dma_start(out=outT[:, 0:NF, :],
                                      in_=y_all[:, 0:NF, :])

                    # ---- o-tile 7: A(g0-15) + B1(g16-23) + B2(g24-31) ----
                    L7 = NOT - 1
                    py7 = mkps(4, "py", 2)
                    rA7 = mkps(NH, "rA", 2)
                    mm_groups(L7, 0, 16, rA7)
                    stamp()
                    nc.scalar.copy(out=raw_all[:, L7, :, 0:16],
                                   in_=psum_tg(rA7, 16))
                    stamp()
                    nc.vector.tensor_tensor(
                        out=scl_all[:, L7, :, 0:16],
                        in0=raw_all[:, L7, :, 0:16],
                        in1=s_bc(L7, 0, 16), op=mul)
                    emit_ymm(L7, 0, 4, py7)

                    rB1 = mkps(8, "rB", 1)
                    mm_groups(L7, 16, 24, rB1)
                    stamp()
                    nc.scalar.copy(out=raw_all[:, L7, :, 16:24],
                                   in_=psum_tg(rB1, 8))
                    stamp()
                    nc.vector.tensor_tensor(
                        out=scl_all[:, L7, :, 16:24],
                        in0=raw_all[:, L7, :, 16:24],
                        in1=s_bc(L7, 16, 8), op=mul)
                    emit_ymm(L7, 4, 6, py7)

                    rB2 = mkps(8, "rB", 1)
                    mm_groups(L7, 24, 32, rB2)
                    stamp()
                    nc.scalar.copy(out=raw_all[:, L7, :, 24:32],
                                   in_=psum_tg(rB2, 8))
                    stamp()
                    nc.vector.tensor_tensor(
                        out=scl_all[:, L7, :, 24:32],
                        in0=raw_all[:, L7, :, 24:32],
                        in1=s_bc(L7, 24, 8), op=mul)
                    emit_ymm(L7, 6, 8, py7)

                    # token-split finish + stores
                    stamp()
                    nc.vector.tensor_reduce(
                        out=tmp_all[:, L7, :],
                        in_=bass.AP(tensor=py7.tensor, offset=py7.offset,
                                    ap=[py7.ap[0], [4, TOKENS], [1, 4]]),
                        axis=X, op=add)
                    for t0, t1 in ((0, TOKENS // 2), (TOKENS // 2, TOKENS)):
                        stamp()
                        nc.vector.tensor_tensor(
                            out=y_all[:, L7, t0:t1],
                            in0=tmp_all[:, L7, t0:t1],
                            in1=scco_sb[:, L7, NG + t0:NG + t1], op=add)
                        stamp()
                        nc.sync.dma_start(out=outT[:, L7, t0:t1],
                                          in_=y_all[:, L7, t0:t1])

    nc.compile()
    names = dict(w8=w8.name, xt=xt.name, scco=scco.name, ident=ident.name,
                 outT=outT.name)
    _cache["nc"] = nc
    _cache["names"] = names
    return nc, names


def _gather_core(outT_host):
    # outT_host[p, ot, t] -> [t, ot*128 + p]
    o = np.asarray(outT_host).astype(np.float32)
    return o.transpose(2, 1, 0).reshape(TOKENS, OC)


def _host_prep(x, weight_packed, scales, bias):
    """Build the 8 per-core input maps."""
    _, names = _build_nc()

    wp = np.ascontiguousarray(weight_packed).view(np.uint32)  # [8192, 1024]
    shifts = (np.arange(8, dtype=np.uint32) * 4)[None, None, :]
    nib = ((wp[:, :, None] >> shifts) & np.uint32(0xF)).astype(np.uint8)
    nib = nib.reshape(OUT_F, IN_F)  # n[o, i]
    lut = np.arange(16, dtype=np.float32).astype(ml_dtypes.float8_e4m3)
    nfp8 = lut[nib]  # [8192, 8192] fp8, exact

    xb = x.astype(ml_dtypes.bfloat16)
    xf = xb.astype(np.float32)
    # xt_host[p, r, t] = x_bf16[t, 128r + p]
    xt_host = np.ascontiguousarray(
        xb.T.reshape(NCHUNK, 128, TOKENS).transpose(1, 0, 2))
    # corr[o, t] = -8 * sum_g s[o,g] * xsum_g[t] + bias[o]
    xsum = xf.reshape(TOKENS, NG, GROUP).sum(axis=2)  # [t, g]
    corr = (-8.0 * scales.astype(np.float64) @ xsum.astype(np.float64).T
            + bias.astype(np.float64)[:, None]).astype(np.float32)  # [8192, 64]
    ident_host = np.eye(128, dtype=ml_dtypes.bfloat16)

    in_maps = []
    for k in range(NCORES):
        osl = slice(OC * k, OC * (k + 1))
        nk = nfp8[osl]  # [1024, 8192]
        # w8_host[p, ot, r, c] = nk[ot*128 + c, 128*r + p]
        w8_host = np.ascontiguousarray(
            nk.reshape(NOT, 128, NCHUNK, 128).transpose(3, 0, 2, 1)
        )
        sck = scales[osl]  # [1024, 32]
        scco_host = np.empty((128, NOT, NG + TOKENS), dtype=ml_dtypes.bfloat16)
        scco_host[:, :, :NG] = sck.reshape(NOT, 128, NG).transpose(1, 0, 2)
        scco_host[:, :, NG:] = corr[osl].reshape(NOT, 128, TOKENS).transpose(
            1, 0, 2)
        in_maps.append({
            names["w8"]: w8_host,
            names["xt"]: xt_host,
            names["scco"]: np.ascontiguousarray(scco_host),
            names["ident"]: ident_host,
        })
    return in_maps


def kernel(x, weight_packed, scales, bias):
    from concourse.bass_utils import run_bass_kernel_spmd

    nc, names = _build_nc()
    in_maps = _host_prep(x, weight_packed, scales, bias)
    res = run_bass_kernel_spmd(nc, in_maps, core_ids=list(range(NCORES)))
    out = np.concatenate(
        [_gather_core(res.results[k][names["outT"]]) for k in range(NCORES)],
        axis=1,
    )  # [64, 8192]
    return np.ascontiguousarray(out)


# revision 13
# speedup vs baseline: 1.2157x; 1.1831x over previous
"""Int4-packed linear (group-quantized, 256-group) on 8 Trainium2 cores.

Column-parallel: each core owns 1024 of 8192 out_features.

Math per core (out^T orientation, o on partitions):
  out[o, t] = sum_g s[o,g] * R_g[o,t] + corr[o,t]
  R_g[o,t]  = sum_{i in g} q[o,i] * x[t,i]        (q in 0..15)
  corr[o,t] = -8*sum_g s[o,g]*xsum_g[t] + bias[o]   (precomputed on host)

Weights ship as fp8e4m3 nibble planes (exact small integers), x as bf16.
Weight DRAM layout is o-tile-major and the Sync HWDGE queue streams
x + weights in consumption order as uniform ~1MB units, which keeps the
DMA ring saturated at ~97% of the per-core HBM roofline; the kernel is
stream-bound in the middle and the last o-tile's combine is the tail.

Group partials accumulate in PSUM as two 16-group half tiles per o-tile so
banks release at half-tile granularity (quartered second half on the last
o-tile to shorten the tail chain). Combine per o-tile:
  GpSimd: stage host-precomputed corr as reduce slice 16 (off-chain)
  ACT   : per-piece copies PSUM fp32 -> SBUF bf16, transposed to [t, g]
  DVE   : mult by bf16 scale broadcast; fold group halves; reduce 17 -> y
"""

import sys

import numpy as np
import ml_dtypes

sys.path.insert(0, "/opt/trn_rl_repo")

import concourse.bass as bass  # noqa: E402
import concourse.mybir as mybir  # noqa: E402
import concourse.tile as tile  # noqa: E402
from concourse import bacc  # noqa: E402

NCORES = 8
TOKENS = 64
IN_F = 8192
OUT_F = 8192
GROUP = 256
OC = OUT_F // NCORES  # 1024 out-features per core
NCHUNK = IN_F // 128  # 64 K-chunks of 128
NG = IN_F // GROUP  # 32 groups
NH = NG // 2  # 16 groups per PSUM half tile
NOT = OC // 128  # 8 o-tiles per core

_cache = {}


def _build_nc():
    if "nc" in _cache:
        return _cache["nc"], _cache["names"]

    f32 = mybir.dt.float32
    bf16 = mybir.dt.bfloat16
    nc = bacc.Bacc(None, target_bir_lowering=False, debug=False)
    with tile.TileContext(nc) as tc:
        with tc.tile_pool(name="dram", bufs=1, space="DRAM") as dram:
            # w8[p, ot, r, c] = nibble_fp8[ot*128 + c, 128*r + p]
            w8 = dram.tile([128, NOT, NCHUNK, 128], mybir.dt.float8e4,
                           kind="ExternalInput")
            xt = dram.tile([128, NCHUNK, TOKENS], bf16, kind="ExternalInput")
            sc = dram.tile([128, NOT, NG], bf16, kind="ExternalInput")
            co = dram.tile([128, NOT, TOKENS], bf16, kind="ExternalInput")
            outT = dram.tile([OC, TOKENS], bf16, kind="ExternalOutput")

            with (
                tc.tile_pool(name="wsb", bufs=1) as wsb,
                tc.tile_pool(name="xsb", bufs=1) as xsb,
                tc.tile_pool(name="small", bufs=1) as small,
                tc.tile_pool(name="cmb", bufs=1) as cmb,
                tc.tile_pool(name="ps", bufs=3, space="PSUM") as ps,
                tc.tile_pool(name="psq", bufs=2, space="PSUM") as psq,
            ):
                w_all = wsb.tile([128, NOT, NCHUNK, 128], mybir.dt.float8e4)
                x_all = xsb.tile([128, NCHUNK, TOKENS], bf16)
                sc_all = small.tile([128, NOT, NG], bf16)
                co_sb = small.tile([128, NOT, TOKENS], bf16, tag="co")
                rs_all = cmb.tile([128, NOT, TOKENS, NG], bf16)
                prod_all = cmb.tile([128, NOT, TOKENS, NG], bf16, tag="pr")
                half_all = cmb.tile([128, NOT, TOKENS, NH + 1], bf16, tag="hf")
                y_all = cmb.tile([128, NOT, TOKENS], bf16, tag="y")

                # the scale vector rides the Scalar HWDGE queue; everything
                # else streams on the Sync queue in consumption order.
                # Uniform ~1MB units maximize stream efficiency -- what
                # matters is when o-tile 0 COMPLETES, not the first matmul.
                nc.scalar.dma_start(out=sc_all[:], in_=sc[:])

                nc.sync.dma_start(out=x_all[:], in_=xt[:])
                nc.sync.dma_start(out=w_all[:, 0, :, :], in_=w8[:, 0, :, :])
                nc.sync.dma_start(out=w_all[:, 1, :, :], in_=w8[:, 1, :, :])
                nc.sync.dma_start(out=co_sb[:], in_=co[:])
                for ot in range(2, NOT - 1):
                    nc.sync.dma_start(out=w_all[:, ot, :, :], in_=w8[:, ot, :, :])
                last = NOT - 1
                nc.sync.dma_start(out=w_all[:, last, 0:32, :], in_=w8[:, last, 0:32, :])
                nc.sync.dma_start(out=w_all[:, last, 32:48, :], in_=w8[:, last, 32:48, :])
                nc.sync.dma_start(out=w_all[:, last, 48:64, :], in_=w8[:, last, 48:64, :])

                with nc.allow_low_precision("bf16 combine, validated vs 2e-2"):
                    for ot in range(NOT):
                        nc.gpsimd.tensor_copy(
                            out=half_all[:, ot, :, NH], in_=co_sb[:, ot, :]
                        )
                    for ot in range(NOT):
                        osl = slice(ot * 128, (ot + 1) * 128)
                        rs = rs_all[:, ot, :, :]
                        # last o-tile: quarter the second half so the final
                        # copy+mult chain after the last matmul is shorter
                        bounds = ([0, 16, 24, 32] if ot == NOT - 1
                                  else [0, 16, 32])
                        pieces = list(zip(bounds[:-1], bounds[1:]))
                        for lo, hi in pieces:
                            npg = hi - lo
                            pool = ps if npg == NH else psq
                            r_ps = pool.tile([128, npg, TOKENS], f32)
                            for gg in range(npg):
                                g = lo + gg
                                nc.tensor.matmul(
                                    r_ps[:, gg, :],
                                    lhsT=w_all[:, ot, 2 * g, :],
                                    rhs=x_all[:, 2 * g, :],
                                    start=True,
                                    stop=False,
                                )
                                nc.tensor.matmul(
                                    r_ps[:, gg, :],
                                    lhsT=w_all[:, ot, 2 * g + 1, :],
                                    rhs=x_all[:, 2 * g + 1, :],
                                    start=False,
                                    stop=(gg == npg - 1),
                                )
                            # ACT: PSUM [o, gg, t] fp32 -> SBUF [o, t, g] bf16
                            # (strided PSUM reads, 32B-burst SBUF writes)
                            r_tg = bass.AP(
                                tensor=r_ps.tensor,
                                offset=r_ps.offset,
                                ap=[r_ps.ap[0], [1, TOKENS], [TOKENS, npg]],
                            )
                            nc.scalar.copy(out=rs[:, :, lo:hi], in_=r_tg)

                        half = half_all[:, ot, :, :]
                        # DVE: prod[o, t, g] = rs * s[o, g], one op per
                        # half so each starts right after its copies land
                        s_ot = sc_all[:, ot, :]
                        prod = prod_all[:, ot, :, :]
                        for lo, hi in ((0, NH), (NH, NG)):
                            s_bcast = bass.AP(
                                tensor=s_ot.tensor,
                                offset=s_ot.offset + lo,
                                ap=[s_ot.ap[0], [0, TOKENS], [1, hi - lo]],
                            )
                            nc.vector.tensor_tensor(
                                out=prod[:, :, lo:hi], in0=rs[:, :, lo:hi],
                                in1=s_bcast,
                                op=mybir.AluOpType.mult,
                            )
                        # DVE: fold group halves, then reduce 16+corr slices
                        nc.vector.tensor_tensor(
                            out=half[:, :, 0:NH], in0=prod[:, :, 0:NH],
                            in1=prod[:, :, NH:NG],
                            op=mybir.AluOpType.add,
                        )
                        y = y_all[:, ot, :]
                        if ot == NOT - 1:
                            # token-split the final reduce + store so the
                            # first half of the output ships while the second
                            # half reduces
                            for t0, t1 in ((0, TOKENS // 2), (TOKENS // 2, TOKENS)):
                                nc.vector.tensor_reduce(
                                    out=y[:, t0:t1], in_=half[:, t0:t1, :],
                                    axis=mybir.AxisListType.X,
                                    op=mybir.AluOpType.add,
                                )
                                nc.sync.dma_start(
                                    out=outT[osl, t0:t1], in_=y[:, t0:t1]
                                )
                        else:
                            nc.vector.tensor_reduce(
                                out=y[:], in_=half[:], axis=mybir.AxisListType.X,
                                op=mybir.AluOpType.add,
                            )
                            nc.sync.dma_start(out=outT[osl, :], in_=y[:])

    nc.compile()
    names = dict(w8=w8.name, xt=xt.name, sc=sc.name, co=co.name,
                 outT=outT.name)
    _cache["nc"] = nc
    _cache["names"] = names
    return nc, names


def _host_prep(x, weight_packed, scales, bias):
    """Build the 8 per-core input maps."""
    _, names = _build_nc()

    wp = np.ascontiguousarray(weight_packed).view(np.uint32)  # [8192, 1024]
    shifts = (np.arange(8, dtype=np.uint32) * 4)[None, None, :]
    nib = ((wp[:, :, None] >> shifts) & np.uint32(0xF)).astype(np.uint8)
    nib = nib.reshape(OUT_F, IN_F)  # n[o, i]
    lut = np.arange(16, dtype=np.float32).astype(ml_dtypes.float8_e4m3)
    nfp8 = lut[nib]  # [8192, 8192] fp8, exact

    xb = x.astype(ml_dtypes.bfloat16)
    xf = xb.astype(np.float32)
    # xt_host[p, r, t] = x_bf16[t, 128r + p]
    xt_host = np.ascontiguousarray(xb.T.reshape(NCHUNK, 128, TOKENS).transpose(1, 0, 2))
    # corr[o, t] = -8 * sum_g s[o,g] * xsum_g[t] + bias[o]
    xsum = xf.reshape(TOKENS, NG, GROUP).sum(axis=2)  # [t, g]
    corr = (-8.0 * scales.astype(np.float64) @ xsum.astype(np.float64).T
            + bias.astype(np.float64)[:, None]).astype(np.float32)  # [8192, 64]

    in_maps = []
    for k in range(NCORES):
        osl = slice(OC * k, OC * (k + 1))
        nk = nfp8[osl]  # [1024, 8192]
        # w8_host[p, ot, r, c] = nk[ot*128 + c, 128*r + p]
        w8_host = np.ascontiguousarray(
            nk.reshape(NOT, 128, NCHUNK, 128).transpose(3, 0, 2, 1)
        )
        sck = scales[osl]  # [1024, 32]
        sc_host = np.ascontiguousarray(
            sck.reshape(NOT, 128, NG).transpose(1, 0, 2)
        ).astype(ml_dtypes.bfloat16)
        co_host = np.ascontiguousarray(
            corr[osl].reshape(NOT, 128, TOKENS).transpose(1, 0, 2)
        ).astype(ml_dtypes.bfloat16)
        in_maps.append({
            names["w8"]: w8_host,
            names["xt"]: xt_host,
            names["sc"]: sc_host,
            names["co"]: co_host,
        })
    return in_maps


def kernel(x, weight_packed, scales, bias):
    from concourse.bass_utils import run_bass_kernel_spmd

    nc, names = _build_nc()
    in_maps = _host_prep(x, weight_packed, scales, bias)
    res = run_bass_kernel_spmd(nc, in_maps, core_ids=list(range(NCORES)))
    outs = [res.results[k][names["outT"]] for k in range(NCORES)]  # [1024, 64] bf16
    out = np.concatenate(
        [np.asarray(o).astype(np.float32).T for o in outs], axis=1
    )  # [64, 8192]
    return np.ascontiguousarray(out)

